# revision 10
# baseline (speedup 1.0000x reference)
"""Trainium2 Bass kernel for nn_C2FTransformerV13 (dense transformer).

Sharding: 2-way data parallel over batch x 4-way tensor parallel
(5 attention heads / core, 640 FFN hidden / core), groups [[0-3],[4-7]],
AllReduce after attention out-proj and after FFN w2.

v3 design (v1 baseline 5.52ms -> v2 3.02ms -> v3):
  - fp16 weights + fp16 activations (h/q/k/v/e/on/ff) and fp16 residual;
    PSUM accumulation f32, rms/softmax stats f32, x*x squares f32r (the
    residual peaks at |x|~7e3 in layer 15, so fp16 squares overflow).
    Sims 8.0e-3 vs the 2e-2 budget; HW measures 7.9e-3.
  - AllReduce payloads fp16, split into sequence halves; surrounding
    compute (qkv / attention / FFN) processed half-by-half so each AR
    overlaps the other half's compute.
  - AR-return DMAs issued on the GpSimd queue right after their
    collective (the Sync queue's weight-DMA backlog was adding ~15us
    of latency per boundary in v2).
  - whole-layer weight DMAs (one transfer per weight class) to cut
    Sync-queue trigger count.
  - exp bias -2 keeps exp outputs in fp16 normal range (scores peak ~4).
  - vector.reciprocal_approx_fast (18-bit) for rms/softmax reciprocals.
"""

import sys

for _p in ("/opt/trn_rl_repo", "/root/.axon_site/_ro/trn_rl_repo"):
    if _p not in sys.path:
        sys.path.insert(0, _p)

import math
import numpy as np

# ---- model dims (hardcoded from the problem spec) ----
B, S = 2, 512
D, H, L = 1280, 20, 16
DK = D // H                      # 64
LAT_C = 112
VOCAB, NPRED = 1024, 10
D_INNER = 2 * D                  # 2560
NUM_BUCKETS, MAX_DIST = 32, 128

N_CORES = 8
TP = 4
GROUPS = [[0, 1, 2, 3], [4, 5, 6, 7]]
HL = H // TP                     # 5 local heads
QS = HL * DK                     # 320 local qkv cols
FS = D_INNER // TP               # 640 local ffn hidden
CS = VOCAB * NPRED // TP         # 2560 local cls rows
P = 128
NK = D // P                      # 10 k-tiles over D
NMQ = 3                          # q (or k) M-tiles (320 -> padded 384)
NMC = CS // P                    # 20 cls M-tiles
EPS = 1e-5
CH = 256                         # pipeline half width
HALVES = (slice(0, CH), slice(CH, S))


# ------------------------------------------------------------------
# host-side prep
# ------------------------------------------------------------------

def _relative_buckets(s):
    ctx = np.arange(s)[:, None]
    mem = np.arange(s)[None, :]
    n = -(mem - ctx)
    nb = NUM_BUCKETS // 2
    ret = (n < 0).astype(np.int64) * nb
    n = np.abs(n)
    max_exact = nb // 2
    is_small = n < max_exact
    safe = np.maximum(n, 1).astype(np.float32)
    val_large = max_exact + (
        np.log(safe / max_exact) / math.log(MAX_DIST / max_exact) * (nb - max_exact)
    ).astype(np.int64)
    val_large = np.minimum(val_large, nb - 1)
    return ret + np.where(is_small, n, val_large)


def _img_kxm(w, nk, nm):
    """[K=nk*128, M=nm*128] -> [128, nm*nk, 128]; tile (m,k) at index m*nk+k."""
    assert w.shape == (nk * P, nm * P)
    return np.ascontiguousarray(
        w.reshape(nk, P, nm, P).transpose(1, 2, 0, 3).reshape(P, nm * nk, P)
    )


def _pad_cols(w, to):
    out = np.zeros((w.shape[0], to), w.dtype)
    out[:, : w.shape[1]] = w
    return out


def _pad_rows(w, to):
    out = np.zeros((to, w.shape[1]), w.dtype)
    out[: w.shape[0]] = w
    return out


def make_in_maps(inputs):
    """inputs: full f32 arrays keyed as in setup_inputs(). Returns in_maps[8]."""
    latents = np.asarray(inputs["latents"], np.float32)
    emb_w = np.asarray(inputs["emb_w"], np.float32)
    emb_b = np.asarray(inputs["emb_b"], np.float32)
    norm_w = np.asarray(inputs["norm_w"], np.float32)
    wq = np.asarray(inputs["wq"], np.float32)
    wk = np.asarray(inputs["wk"], np.float32)
    wv = np.asarray(inputs["wv"], np.float32)
    wo = np.asarray(inputs["wo"], np.float32)
    w1 = np.asarray(inputs["w1"], np.float32)
    w2 = np.asarray(inputs["w2"], np.float32)
    rel_bias = np.asarray(inputs["rel_bias"], np.float32)
    norm_out_w = np.asarray(inputs["norm_out_w"], np.float32)
    cls_w = np.asarray(inputs["cls_w"], np.float32)
    cls_b = np.asarray(inputs["cls_b"], np.float32)

    pb = rel_bias[_relative_buckets(S)]        # [sq, sk, H]

    in_maps = []
    rank_cache = []
    for r in range(TP):
        qc = slice(r * QS, (r + 1) * QS)
        fc = slice(r * FS, (r + 1) * FS)
        gc = slice(D_INNER + r * FS, D_INNER + (r + 1) * FS)
        cc = slice(r * CS, (r + 1) * CS)

        wqk_l, wv_l, wo_l, w1_l, w2_l = [], [], [], [], []
        for l in range(L):
            nw = norm_w[l][:, None]
            wq_eff = (nw * wq[l] / math.sqrt(DK))[:, qc]      # [1280, 320]
            wk_eff = (nw * wk[l])[:, qc]
            wv_eff = (nw * wv[l])[:, qc]
            q_img = _img_kxm(_pad_cols(wq_eff, NMQ * P), NK, NMQ)
            k_img = _img_kxm(_pad_cols(wk_eff, NMQ * P), NK, NMQ)
            wqk_l.append(np.concatenate([q_img, k_img], axis=1))  # [128,60,128]
            wv_l.append(
                np.ascontiguousarray(wv_eff.reshape(NK, P, QS).transpose(1, 0, 2))
            )  # [128, 10, 320]
            wo_l.append(_img_kxm(_pad_rows(wo[l][qc, :], 3 * P), 3, NK))
            w1_sh = np.concatenate([w1[l][:, fc], w1[l][:, gc]], axis=1)
            w1_l.append(_img_kxm(w1_sh, NK, NK))                # [128,100,128]
            w2_l.append(_img_kxm(w2[l][fc.start : fc.stop, :], 5, NK))

        cls_eff = (cls_w[cc, :] * norm_out_w[None, :]).T        # [1280, 2560]
        cls_img = _img_kxm(cls_eff, NK, NMC)                    # [128, 200, 128]
        clsb_img = np.ascontiguousarray(cls_b[cc].reshape(NMC, P).T)

        posb = np.ascontiguousarray(
            pb[:, :, r * HL : (r + 1) * HL].transpose(2, 1, 0)  # [5, sk, sq]
        ).reshape(HL, 4, P, S)

        rank_cache.append(
            dict(
                wqk=np.stack(wqk_l).astype(np.float16),
                wv=np.stack(wv_l).astype(np.float16),
                wo=np.stack(wo_l).astype(np.float16),
                w1=np.stack(w1_l).astype(np.float16),
                w2=np.stack(w2_l).astype(np.float16),
                cls=cls_img.astype(np.float16),
                clsb=clsb_img.astype(np.float32),
                posb=posb.astype(np.float16),
            )
        )

    embw_img = np.ascontiguousarray(emb_w.T).astype(np.float16)   # [112, 1280]
    embb_img = np.ascontiguousarray(emb_b.reshape(NK, P).T).astype(np.float32)

    for c in range(N_CORES):
        g, r = divmod(c, TP)
        m = dict(rank_cache[r])
        m["embw"] = embw_img
        m["embb"] = embb_img
        m["lat"] = latents[g].astype(np.float16)                  # [112, 512]
        in_maps.append(m)
    return in_maps


# ------------------------------------------------------------------
# device kernel
# ------------------------------------------------------------------

_BUILD_CACHE = {}


def build_nc():
    if "nc" in _BUILD_CACHE:
        return _BUILD_CACHE["nc"]

    import contextlib

    import concourse.mybir as mybir
    import concourse.tile as tile
    from concourse import bacc

    dt = mybir.dt
    AF = mybir.ActivationFunctionType
    f32, f32r, f16 = dt.float32, dt.float32r, dt.float16

    nc = bacc.Bacc("TRN2", target_bir_lowering=False, debug=False,
                   num_devices=N_CORES)

    wqk_d = nc.dram_tensor("wqk", [L, P, 60, P], f16, kind="ExternalInput")
    wv_d = nc.dram_tensor("wv", [L, P, NK, QS], f16, kind="ExternalInput")
    wo_d = nc.dram_tensor("wo", [L, P, 30, P], f16, kind="ExternalInput")
    w1_d = nc.dram_tensor("w1", [L, P, 100, P], f16, kind="ExternalInput")
    w2_d = nc.dram_tensor("w2", [L, P, 50, P], f16, kind="ExternalInput")
    cls_d = nc.dram_tensor("cls", [P, 200, P], f16, kind="ExternalInput")
    clsb_d = nc.dram_tensor("clsb", [P, NMC], f32, kind="ExternalInput")
    embw_d = nc.dram_tensor("embw", [LAT_C, D], f16, kind="ExternalInput")
    embb_d = nc.dram_tensor("embb", [P, NK], f32, kind="ExternalInput")
    lat_d = nc.dram_tensor("lat", [LAT_C, S], f16, kind="ExternalInput")
    posb_d = nc.dram_tensor("posb", [HL, 4, P, S], f16, kind="ExternalInput")

    out_d = nc.dram_tensor("out", [NMC, P, S], f32, kind="ExternalOutput")

    with tile.TileContext(nc) as tc:
        ctx = contextlib.ExitStack()
        with ctx:
            sp = lambda name, bufs: ctx.enter_context(
                tc.tile_pool(name=name, bufs=bufs)
            )
            static = sp("static", 1)
            x_sb = static.tile([P, NK, S], f16, name="x_sb")
            h16 = static.tile([P, NK, S], f16, name="h16")
            q16 = static.tile([P, NMQ, S], f16, name="q16")
            k16 = static.tile([P, NMQ, S], f16, name="k16")
            v16 = static.tile([P, 4, HL * 65], f16, name="v16")
            on16 = static.tile([P, NMQ, S], f16, name="on16")
            ff16 = static.tile([P, 5, S], f16, name="ff16")
            emb_sb = static.tile([LAT_C, D], f16, name="emb_sb")
            lat_sb = static.tile([LAT_C, S], f16, name="lat_sb")
            embb_sb = static.tile([P, NK], f32, name="embb_sb")
            clsb_sb = static.tile([P, NMC], f32, name="clsb_sb")
            ones_c = static.tile([P, 1], f32r, name="ones_c")
            ones_r = static.tile([1, P], f32r, name="ones_r")
            eps_t = static.tile([1, 1], f32, name="eps_t")
            ebias_t = static.tile([P, 1], f32, name="ebias_t")

            wqk_p = sp("wqk_p", 1)      # [128,60,128] whole layer
            wv_p = sp("wv_p", 2)
            wo_p = sp("wo_p", 1)        # [128,30,128]
            w1_p = sp("w1_p", 1)        # [128,100,128]
            w2_p = sp("w2_p", 1)        # [128,50,128]
            clsw_p = sp("clsw_p", 4)
            part_p = sp("part_p", 2)    # wo partial staging (fp16)
            part2_p = sp("part2_p", 2)  # w2 partial staging
            art_p = sp("art_p", 4)      # AR returns
            sc_p = sp("sc_p", 6)        # [1,CH] stats rows
            rb_p = sp("rb_p", 2)        # [64,CH] recip-bcast
            e_p = sp("e_p", 4)          # exp outputs
            pb_p = sp("pb_p", 3)        # pos bias (layer 0)
            ga_p = sp("ga_p", 2)        # gelu(g)
            sq_p = sp("sq_p", 3)        # x*x scratch
            osc_p = sp("osc_p", 2)      # odd-head attn staging
            outw_p = sp("outw_p", 2)    # cls output staging

            ps = ctx.enter_context(tc.tile_pool(name="ps", bufs=1, space="PSUM"))
            dram = ctx.enter_context(tc.tile_pool(name="dram", bufs=2,
                                                  space="DRAM"))

            def acc_tile(n=CH, bufs=3):
                return ps.tile([P, n], f32, tag="acc", bufs=bufs,
                               padded_shape=[P, 512], name="acc")

            nc.vector.memset(ones_c[:].bitcast(f32), 1.0)
            nc.vector.memset(ones_r[:].bitcast(f32), 1.0)
            nc.vector.memset(eps_t[:], EPS)
            nc.vector.memset(ebias_t[:], -2.0)
            # ones column per head in v (softmax denominator rows);
            # zero the padded on16 region once (wo pad weights are zero,
            # but NaN garbage * 0 = NaN).
            nc.vector.memset(
                v16.rearrange("p c (h e) -> p c h e", e=65)[:, :, :, 64:65], 1.0)
            nc.vector.memset(on16[64:, NMQ - 1, :], 0.0)

            nc.sync.dma_start(emb_sb[:], embw_d.ap())
            nc.sync.dma_start(lat_sb[:], lat_d.ap())
            nc.sync.dma_start(embb_sb[:], embb_d.ap())
            nc.sync.dma_start(clsb_sb[:], clsb_d.ap())

            # ---------------- embedding ----------------
            for m in range(NK):
                for cols in HALVES:
                    acc = acc_tile()
                    nc.tensor.matmul(acc[:], emb_sb[:, m * P : (m + 1) * P],
                                     lat_sb[:, cols], start=True, stop=True)
                    nc.vector.tensor_scalar_add(x_sb[:, m, cols], acc[:],
                                                embb_sb[:, m : m + 1])

            def rms_half(cols, dst):
                """dst[:, :, cols] = x / rms(x) as fp16."""
                ssp = ps.tile([1, CH], f32, tag="s", bufs=2,
                              padded_shape=[1, 512], name="ssp")
                for j in range(NK):
                    sq = sq_p.tile([P, CH], f32r, tag="sq")
                    nc.vector.tensor_mul(sq[:], x_sb[:, j, cols],
                                         x_sb[:, j, cols])
                    nc.tensor.matmul(ssp[:], ones_c[:], sq[:],
                                     start=(j == 0), stop=(j == NK - 1))
                srt = sc_p.tile([1, CH], f32, tag="sc")
                nc.scalar.activation(srt[:], ssp[:], AF.Sqrt,
                                     bias=eps_t[:], scale=1.0 / D)
                rcp = sc_p.tile([1, CH], f32, tag="sc")
                nc.vector.reciprocal_approx_fast(rcp[:], srt[:])
                rcpr = sc_p.tile([1, CH], f32r, tag="sc")
                nc.scalar.activation(rcpr[:], rcp[:], AF.Copy)
                bc = ps.tile([P, CH], f32, tag="n", bufs=1,
                             padded_shape=[P, 512], name="bc")
                nc.tensor.matmul(bc[:], ones_r[:], rcpr[:],
                                 start=True, stop=True)
                for j in range(NK):
                    nc.vector.tensor_mul(dst[:, j, cols], x_sb[:, j, cols],
                                         bc[:])

            def ar_launch(part, tag):
                """Store partial to HBM, AllReduce (fp16), and queue the
                return fetch on the GpSimd queue (right behind its own
                collective, so it fires the moment the AR completes,
                skipping the Sync queue's weight-DMA backlog).  Returns
                the SBUF tile the result will land in."""
                cci = dram.tile([NK, P, CH], f16, tag=f"{tag}i", bufs=2,
                                name=f"{tag}i")
                cco = dram.tile([NK, P, CH], f16, tag=f"{tag}o", bufs=2,
                                name=f"{tag}o")
                nc.sync.dma_start(cci[:].rearrange("t p n -> p t n"), part[:])
                nc.gpsimd.collective_compute(
                    "AllReduce", mybir.AluOpType.add, replica_groups=GROUPS,
                    ins=[cci[:]], outs=[cco[:]],
                )
                art = art_p.tile([P, NK, CH], f16, tag="art", name=f"{tag}a")
                nc.gpsimd.dma_start(art[:], cco[:].rearrange("t p n -> p t n"))
                return art

            def ar_apply(art, cols):
                nc.vector.tensor_add(x_sb[:, :, cols], x_sb[:, :, cols],
                                     art[:])

            # ---------------- layers ----------------
            art2 = [None, None]         # pending FFN AR results
            for l in range(L):
                # whole-layer weight loads (single transfers)
                wv_t = wv_p.tile([P, NK, QS], f16, tag="wv", name=f"wv_{l}")
                nc.sync.dma_start(wv_t[:], wv_d.ap()[l])
                wqk_t = wqk_p.tile([P, 60, P], f16, tag="wqk", name=f"wqk_{l}")
                nc.sync.dma_start(wqk_t[:], wqk_d.ap()[l])
                wo_t = wo_p.tile([P, 30, P], f16, tag="wo", name=f"wo_{l}")
                nc.sync.dma_start(wo_t[:], wo_d.ap()[l])

                # -- per half: finalize x, rmsnorm, q/k/v projections --
                for hf, cols in enumerate(HALVES):
                    if art2[hf] is not None:
                        ar_apply(art2[hf], cols)
                    rms_half(cols, h16)
                    for m in range(2 * NMQ):
                        acc = acc_tile()
                        for k in range(NK):
                            nc.tensor.matmul(acc[:],
                                             wqk_t[:, m * NK + k, :],
                                             h16[:, k, cols],
                                             start=(k == 0), stop=(k == NK - 1))
                        dst = q16 if m < NMQ else k16
                        nc.vector.tensor_copy(dst[:, m % NMQ, cols], acc[:])
                    for cc in (2 * hf, 2 * hf + 1):
                        vacc = ps.tile([P, QS], f32, tag="av", bufs=2,
                                       padded_shape=[P, 512], name="vacc")
                        for k in range(NK):
                            nc.tensor.matmul(vacc[:],
                                             h16[:, k, cc * P : (cc + 1) * P],
                                             wv_t[:, k, :],
                                             start=(k == 0), stop=(k == NK - 1))
                        nc.vector.tensor_copy(
                            v16.rearrange("p c (h e) -> p c h e", e=65)
                            [:, cc, :, :64],
                            vacc[:].rearrange("p (h e) -> p h e", e=DK),
                        )

                # -- attention + out-proj per q-half; AR1 per half --
                art1 = [None, None]
                for hf, cols in enumerate(HALVES):
                    for pair in ((0, 1), (2, 3), (4,)):
                        av = {}
                        for hh in pair:
                            av[hh] = ps.tile([65, CH], f32, tag="av", bufs=2,
                                             padded_shape=[65, 512],
                                             name=f"av_{l}_{hf}_{hh}")
                        for cch in range(4):
                            for hh in pair:
                                base = (hh % 2) * 64
                                t = hh // 2
                                s_ps = ps.tile([P, CH], f32, tag="s", bufs=2,
                                               padded_shape=[P, 512],
                                               name="s_ps")
                                nc.tensor.matmul(
                                    s_ps[:],
                                    k16[base : base + DK, t,
                                        cch * P : (cch + 1) * P],
                                    q16[base : base + DK, t, cols],
                                    start=True, stop=True,
                                )
                                if l == 0:
                                    pbt = pb_p.tile([P, CH], f16, tag="pb")
                                    nc.sync.dma_start(
                                        pbt[:], posb_d.ap()[hh, cch, :, cols])
                                    nc.vector.tensor_add(s_ps[:], s_ps[:],
                                                         pbt[:])
                                e_t = e_p.tile([P, CH], f16, tag="e")
                                nc.scalar.activation(e_t[:], s_ps[:], AF.Exp,
                                                     bias=ebias_t[:])
                                nc.tensor.matmul(
                                    av[hh][:],
                                    v16[:, cch, hh * 65 : hh * 65 + 65],
                                    e_t[:],
                                    start=(cch == 0), stop=(cch == 3),
                                )
                        for hh in pair:
                            base = (hh % 2) * 64
                            t = hh // 2
                            den = sc_p.tile([1, CH], f32, tag="sc")
                            nc.vector.tensor_copy(den[:], av[hh][64:65, :])
                            rcp = sc_p.tile([1, CH], f32, tag="sc")
                            nc.vector.reciprocal_approx_fast(rcp[:], den[:])
                            rcpr = sc_p.tile([1, CH], f32r, tag="sc")
                            nc.scalar.activation(rcpr[:], rcp[:], AF.Copy)
                            rb_ps = ps.tile([64, CH], f32, tag="acc", bufs=3,
                                            padded_shape=[64, 512],
                                            name="rb_ps")
                            nc.tensor.matmul(rb_ps[:], ones_r[:, :64], rcpr[:],
                                             start=True, stop=True)
                            rb = rb_p.tile([64, CH], f16, tag="rb")
                            nc.vector.tensor_copy(rb[:], rb_ps[:])
                            if base == 0:
                                nc.vector.tensor_mul(on16[:64, t, cols],
                                                     av[hh][:64, :], rb[:])
                            else:
                                osc = osc_p.tile([64, CH], f16, tag="osc")
                                nc.vector.tensor_mul(osc[:], av[hh][:64, :],
                                                     rb[:])
                                nc.sync.dma_start(on16[64:, t, cols], osc[:])

                    part = part_p.tile([P, NK, CH], f16, tag="part",
                                       name=f"part_{l}_{hf}")
                    for m in range(NK):
                        acc = acc_tile()
                        for k3 in range(3):
                            nc.tensor.matmul(acc[:], wo_t[:, m * 3 + k3, :],
                                             on16[:, k3, cols],
                                             start=(k3 == 0), stop=(k3 == 2))
                        nc.vector.tensor_copy(part[:, m, :], acc[:])
                    art1[hf] = ar_launch(part, "cc1")

                # prefetch ffn weights (single transfers)
                w1_t = w1_p.tile([P, 100, P], f16, tag="w1", name=f"w1_{l}")
                nc.sync.dma_start(w1_t[:], w1_d.ap()[l])
                w2_t = w2_p.tile([P, 50, P], f16, tag="w2", name=f"w2_{l}")
                nc.sync.dma_start(w2_t[:], w2_d.ap()[l])

                # -- per half: residual, gated-GELU FFN, AR2 --
                for hf, cols in enumerate(HALVES):
                    ar_apply(art1[hf], cols)
                    for m in range(5):
                        a_ps = acc_tile()
                        for k in range(NK):
                            nc.tensor.matmul(a_ps[:], w1_t[:, m * NK + k, :],
                                             x_sb[:, k, cols],
                                             start=(k == 0), stop=(k == NK - 1))
                        g_ps = acc_tile()
                        for k in range(NK):
                            nc.tensor.matmul(g_ps[:],
                                             w1_t[:, (m + 5) * NK + k, :],
                                             x_sb[:, k, cols],
                                             start=(k == 0), stop=(k == NK - 1))
                        ga = ga_p.tile([P, CH], f16, tag="ga")
                        nc.scalar.activation(ga[:], g_ps[:],
                                             AF.Gelu_apprx_tanh)
                        nc.vector.tensor_mul(ff16[:, m, cols], a_ps[:], ga[:])
                    part2 = part2_p.tile([P, NK, CH], f16, tag="part2",
                                         name=f"part2_{l}_{hf}")
                    for m in range(NK):
                        acc = acc_tile()
                        for k5 in range(5):
                            nc.tensor.matmul(acc[:], w2_t[:, m * 5 + k5, :],
                                             ff16[:, k5, cols],
                                             start=(k5 == 0), stop=(k5 == 4))
                        nc.vector.tensor_copy(part2[:, m, :], acc[:])
                    art2[hf] = ar_launch(part2, "cc2")

            # ---------------- final norm + classifier ----------------
            for hf, cols in enumerate(HALVES):
                ar_apply(art2[hf], cols)
                rms_half(cols, h16)
                for m in range(NMC):
                    cw = clsw_p.tile([P, NK, P], f16, tag="clsw")
                    nc.sync.dma_start(
                        cw[:], cls_d.ap()[:, m * NK : (m + 1) * NK, :])
                    acc = acc_tile()
                    for k in range(NK):
                        nc.tensor.matmul(acc[:], cw[:, k, :], h16[:, k, cols],
                                         start=(k == 0), stop=(k == NK - 1))
                    ot = outw_p.tile([P, CH], f32, tag="outw")
                    nc.vector.tensor_scalar_add(ot[:], acc[:],
                                                clsb_sb[:, m : m + 1])
                    nc.sync.dma_start(out_d.ap()[m, :, cols], ot[:])

    nc.compile()
    _BUILD_CACHE["nc"] = nc
    return nc


def _run(in_maps, **kw):
    from concourse import bass_utils

    nc = build_nc()
    return bass_utils.run_bass_kernel_spmd(
        nc, in_maps, core_ids=list(range(N_CORES)), **kw
    )


def kernel(**inputs):
    in_maps = make_in_maps(inputs)
    res = _run(in_maps)
    return assemble_output(res.results)


def assemble_output(results):
    full = np.empty((B, VOCAB * NPRED, S), np.float32)
    for c in range(N_CORES):
        g, r = divmod(c, TP)
        full[g, r * CS : (r + 1) * CS] = (
            np.asarray(results[c]["out"], np.float32).reshape(CS, S)
        )
    out = (
        full.reshape(B, VOCAB, NPRED, S)
        .transpose(0, 1, 3, 2)
        .reshape(B, VOCAB, S * NPRED)
    )
    return np.ascontiguousarray(out)
